# revision 29
# baseline (speedup 1.0000x reference)
"""Trainium2 Bass kernel for nn_AvgPoolVectorsPerWSI (segment-mean over groups).

Math: x [N=2048, M=512, 7, 7], idx [N] in [0,64)
  out[g, m] = mean over {n: idx[n]==g} and spatial of x[n, m, :, :]  -> [64, 512, 1, 1]

Strategy (no collectives needed):
  - Shard over M: core k handles an m-slice of 64 channels. Each core
    streams its x slice [2048, 64, 49] (25.7 MB) once at fabric line rate
    (~430 GB/s, 466 ns per 12.5 KB row descriptor); the ~62 us stream is
    the floor and everything else hides behind it. 16 n-tiles of 128 rows
    (128-partition DMAs only -- partition-partial bulk DMAs mis-lower).
  - Per tile: PE accumulates the first MC=10 channels raw
    (psum_raw[g, (m,j)] += w^T @ x, one 490-col fp32 chunk), DVE spatially
    j-reduces the other 54 channels ([128, 54, 49] -> [128, 54],
    ~2.9 us/tile vs ~3.9 us/tile DMA pace -- real headroom), then PE adds
    two small fp32 matmuls psum[g, m] += w^T @ xs, split m[10:46]/m[46:64]
    so only a sliver depends on the last DMA piece. w is the
    scale-weighted one-hot (scale = 1/(count_g*49)) generated ON DEVICE
    from a 74 KB aux tensor loaded first on the ring (w-gen and the whole
    DVE chain hang off it).
  - Tail minimization (stream end -> block end is what matters; the ~7 us
    NRT postamble after it is fixed):
      * tile 15 arrives in 4 column pieces (raw chunk, then 3x18 channels)
        so the last exposed reduce is ~1 us;
      * PSUM->SBUF copies run on ScalarE, warmed up at start (the first
        activation op lazily loads the ACT table, ~1.3 us);
      * DVE j-reduces psum_raw into out[:, 0:10] concurrently with the
        final matmul sliver;
      * the final out DMA has no completion wait -- NRT's postamble
        (sem resets, then dma_rearm) drains the ring with ~5 us margin.

Raw Block implementation (not Tile): the walrus matmul/DMA lowerings only
accept ONE attached sync-wait per instruction; standalone wait_ge
instructions sidestep that.

PSUM: a matmul with start=True clears the has_written bits of its WHOLE
bank, so the three accumulation streams (raw / half A / half B) live in
three separate psum tensors.

Known environmental hazard: SDMA engine 15 episodically runs ~20-25% slow,
and every tile's completion semaphore needs all 16 engines, so on such
runs the stream is engine-15-paced (~+14 us) regardless of kernel
structure; partition-exclusion workarounds mis-lower (see above), so this
is accepted as run-to-run variance.
"""

from contextlib import ExitStack

import numpy as np

import concourse.bass as bass
import concourse.mybir as mybir
from concourse.bass_utils import run_bass_kernel_spmd

N = 2048          # samples
M = 512           # channels
HW = 49           # spatial (7*7)
G = 64            # groups
CORES = 8
ML = M // CORES   # 64 channels per core
F = ML * HW       # 3136 floats per (n, core)
P = 128           # partitions per tile
NT = N // P       # 16 n-tiles
BUFS = 8          # x slot ring depth == number of DMA semaphores
MC = 10           # channels on the PE raw path
FC = MC * HW      # 490 raw columns (single PSUM bank)
MV = ML - MC      # 54 channels on the DVE reduce path
# tile 15 pieces: raw chunk [0:FC], then 6 blocks of 9 channels -- fine
# enough that DVE keeps pace with a slow engine's piece deliveries and the
# last exposed reduce is ~0.6 us
PIECE_CH = [(MC, 19), (19, 28), (28, 37), (37, 46), (46, 55), (55, 64)]
NP = 1 + len(PIECE_CH)  # piece DMAs incl. the raw chunk
MB = 55           # matmul half A covers m[MC:55]; half B covers m[55:64]

F32 = mybir.dt.float32


def _build():
    nc = bass.Bass(trn_type="TRN2", target_bir_lowering=False)
    x_ext = nc.declare_dram_parameter("x", [N, F], F32, isOutput=False)
    # aux[:, 0:64] iota row, aux[:, 64:128] scale row, aux[:, 128:144] idx
    aux_ext = nc.declare_dram_parameter("aux", [P, G + G + NT], F32,
                                        isOutput=False)
    out_ext = nc.declare_dram_parameter("out", [G, ML], F32, isOutput=True)

    x_t = x_ext.ap().rearrange("(t p) f -> t p f", p=P)  # [16, 128, 3136]

    with ExitStack() as ctx:
        x_buf = ctx.enter_context(nc.sbuf_tensor([P, BUFS * F], F32))
        xs_buf = ctx.enter_context(nc.sbuf_tensor([P, BUFS * MV], F32))
        aux_sb = ctx.enter_context(nc.sbuf_tensor([P, G + G + NT], F32))
        w_sb = ctx.enter_context(nc.sbuf_tensor([P, NT * G], F32))
        out_sb = ctx.enter_context(nc.sbuf_tensor([G, ML], F32))
        warm_sb = ctx.enter_context(nc.sbuf_tensor([P, 1], F32))
        # separate tensors -> separate PSUM accumulation state
        psum_raw = ctx.enter_context(nc.psum_tensor([G, FC], F32))
        psum_a = ctx.enter_context(nc.psum_tensor([G, MB - MC], F32))
        psum_b = ctx.enter_context(nc.psum_tensor([G, ML - MB], F32))
        dma_x = [
            ctx.enter_context(nc.semaphore(name=f"dma_x{s}"))
            for s in range(BUFS)
        ]
        # one dedicated sem per tile-15 piece: cumulative thresholds on a
        # shared sem alias across outstanding transfers (a straggling SDMA
        # engine's missing stripes can be masked by other engines' incs
        # from later pieces), so each piece gets a single-use sem
        dma_p = [
            ctx.enter_context(nc.semaphore(name=f"dma_p{p}"))
            for p in range(NP)
        ]
        # tile 0's second row-half (its own sem: two transfers may be
        # outstanding at once, and only an exact-total threshold is
        # alias-safe)
        dma_h = ctx.enter_context(nc.semaphore(name="dma_h"))
        dma_a = ctx.enter_context(nc.semaphore())   # +16 when aux resident
        dma_o = ctx.enter_context(nc.semaphore())   # out DMA (never waited)
        wg_sem = ctx.enter_context(nc.semaphore())  # +1 when w generated
        red_sem = ctx.enter_context(nc.semaphore())  # +1 per DVE reduce op
        pe_sem = ctx.enter_context(nc.semaphore())   # +1 per PE matmul
        fin_sem = ctx.enter_context(nc.semaphore())  # +1 per out_sb writer
        block = ctx.enter_context(nc.Block())

        # cumulative dma_x thresholds: slot s serves tiles s and s+8;
        # tile 15 (slot 7) arrives as 4 pieces of +16 each (32/48/64/80)
        def xthresh(t):
            return 16 * (t // BUFS) + 16

        # pe_sem incs: 3 per tile (raw, mm_a, mm_b)
        def pe_count(t):
            return 3 * (t + 1)

        # ---- x-stream DMA program (SP / HWDGE ring, FIFO) ----
        @block.sync
        def _(sync):
            # tile 0 in two row-halves: 64 descriptors generate in half the
            # time, so the first HBM byte (and the whole stream) moves up
            sync.dma_start(
                out=x_buf[0:64, 0:F], in_=x_t[0][0:64, :]
            ).then_inc(dma_x[0], 16)
            sync.dma_start(
                out=x_buf[64:128, 0:F], in_=x_t[0][64:128, :]
            ).then_inc(dma_h, 16)
            # aux after tile 0: its descriptor gen overlaps tile 0's
            # stream instead of delaying the first x byte; the hybrid's
            # DVE slack absorbs the later w-gen start
            sync.dma_start(out=aux_sb[:, :], in_=aux_ext.ap()).then_inc(
                dma_a, 16
            )
            for t in range(1, NT - 1):
                if t >= BUFS:
                    # slot reuse: all three matmuls of tile t-8 are done
                    # (mm_b implies the DVE reduce consumed the slot too)
                    sync.wait_ge(pe_sem, pe_count(t - BUFS))
                slot = t % BUFS
                sync.dma_start(
                    out=x_buf[:, slot * F:(slot + 1) * F], in_=x_t[t]
                ).then_inc(dma_x[slot], 16)
            # tile 15 -> slot 7, 4 column pieces
            sync.wait_ge(pe_sem, pe_count(NT - 1 - BUFS))
            bounds = [0, FC] + [hi * HW for _, hi in PIECE_CH]
            for p in range(NP):
                lo, hi = bounds[p], bounds[p + 1]
                sync.dma_start(
                    out=x_buf[:, 7 * F + lo:7 * F + hi],
                    in_=x_t[NT - 1][:, lo:hi],
                ).then_inc(dma_p[p], 16)
            # (the out DMA is issued from the ACT queue right after its
            # copies -- saves a cross-engine semaphore hop)

        # ---- VectorE: w generation, spatial j-reduction, raw epilogue ----
        @block.vector
        def _(vector):
            # w[p, t*G+g] = (idx[t*128+p] == g) * scale[g]
            vector.wait_ge(dma_a, 16)
            for t in range(NT):
                wg = vector.scalar_tensor_tensor(
                    out=w_sb[:, t * G:(t + 1) * G],
                    in0=aux_sb[:, 0:G],
                    scalar=aux_sb[:, 2 * G + t:2 * G + t + 1],
                    in1=aux_sb[:, G:2 * G],
                    op0=mybir.AluOpType.is_equal,
                    op1=mybir.AluOpType.mult,
                )
            wg.then_inc(wg_sem, 1)

            for t in range(NT - 1):
                if t >= BUFS:
                    # xs slot reuse: tile t-8's matmuls consumed it
                    vector.wait_ge(pe_sem, pe_count(t - BUFS))
                vector.wait_ge(dma_x[t % BUFS], xthresh(t))
                if t == 0:
                    vector.wait_ge(dma_h, 16)
                slot = t % BUFS
                vector.tensor_reduce(
                    out=xs_buf[:, slot * MV:(slot + 1) * MV],
                    in_=x_buf[:, slot * F + FC:(slot + 1) * F]
                    .rearrange("p (m j) -> p m j", j=HW),
                    axis=mybir.AxisListType.X,
                    op=mybir.AluOpType.add,
                ).then_inc(red_sem, 1)
            # tile 15 (slot 7): 3 piece reduces of 18 channels
            vector.wait_ge(pe_sem, pe_count(NT - 1 - BUFS))
            for p, (clo, chi) in enumerate(PIECE_CH):
                vector.wait_ge(dma_p[p + 1], 16)
                vector.tensor_reduce(
                    out=xs_buf[:, 7 * MV + (clo - MC):7 * MV + (chi - MC)],
                    in_=x_buf[:, 7 * F + clo * HW:7 * F + chi * HW]
                    .rearrange("p (m j) -> p m j", j=HW),
                    axis=mybir.AxisListType.X,
                    op=mybir.AluOpType.add,
                ).then_inc(red_sem, 1)
            # epilogue: j-reduce the raw-path PSUM into out[:, 0:MC]
            vector.wait_ge(pe_sem, pe_count(NT - 2) + 1)  # raw(15) done
            vector.tensor_reduce(
                out=out_sb[:, 0:MC],
                in_=psum_raw[:, :].rearrange("p (m j) -> p m j", j=HW),
                axis=mybir.AxisListType.X,
                op=mybir.AluOpType.add,
            ).then_inc(fin_sem, 1)

        # ---- TensorE: raw chunk + two xs matmuls per tile (fp32) ----
        @block.tensor
        def _(tensor):
            tensor.wait_ge(wg_sem, 1)
            for t in range(NT):
                slot = t % BUFS
                wt = w_sb[:, t * G:(t + 1) * G]
                first, last = (t == 0), (t == NT - 1)
                # raw chunk needs only the tile's first piece in DMA order
                if last:
                    tensor.wait_ge(dma_p[0], 16)
                else:
                    tensor.wait_ge(dma_x[slot], xthresh(t))
                    if first:
                        tensor.wait_ge(dma_h, 16)
                tensor.matmul(
                    out=psum_raw[:, :],
                    lhsT=wt,
                    rhs=x_buf[:, slot * F:slot * F + FC],
                    start=first,
                    stop=last,
                ).then_inc(pe_sem, 1)
                # xs halves; tile 15's reduces land piecewise: half A
                # needs all but the last piece, half B needs the last
                tensor.wait_ge(
                    red_sem, (t + 1) + (len(PIECE_CH) - 2 if last else 0)
                )
                tensor.matmul(
                    out=psum_a[:, :],
                    lhsT=wt,
                    rhs=xs_buf[:, slot * MV:slot * MV + (MB - MC)],
                    start=first,
                    stop=last,
                ).then_inc(pe_sem, 1)
                if last:
                    tensor.wait_ge(red_sem, (t + 1) + len(PIECE_CH) - 1)
                tensor.matmul(
                    out=psum_b[:, :],
                    lhsT=wt,
                    rhs=xs_buf[:, slot * MV + (MB - MC):(slot + 1) * MV],
                    start=first,
                    stop=last,
                ).then_inc(pe_sem, 1)

        # ---- ScalarE (ACT): PSUM -> SBUF copies ----
        @block.scalar
        def _(scalar):
            # warmup: the first activation op lazily DMAs the ACT function
            # table (~1.3 us); trigger it off the critical path
            scalar.activation(
                out=warm_sb[:, :],
                in_=warm_sb[:, :],
                func=mybir.ActivationFunctionType.Copy,
            )
            scalar.wait_ge(pe_sem, pe_count(NT - 1) - 1)  # mm_a(15)
            scalar.activation(
                out=out_sb[:, MC:MB],
                in_=psum_a[:, :],
                func=mybir.ActivationFunctionType.Copy,
            )
            scalar.wait_ge(pe_sem, pe_count(NT - 1))      # mm_b(15)
            scalar.activation(
                out=out_sb[:, MB:ML],
                in_=psum_b[:, :],
                func=mybir.ActivationFunctionType.Copy,
            )
            # out: issue right here (program order covers the two copies;
            # fin covers DVE's raw j-reduce) and do NOT wait -- NRT's
            # postamble drains the ring before rearm
            scalar.wait_ge(fin_sem, 1)
            scalar.dma_start(out=out_ext.ap(), in_=out_sb[:, :]).then_inc(
                dma_o, 16
            )

    return nc


def _prepare(x, idx):
    x = np.asarray(x)
    if x.dtype != np.float32:
        x = x.astype(np.float32)
    idx = np.asarray(idx).astype(np.int64)
    counts = np.bincount(idx, minlength=G).astype(np.float64)
    scale = np.where(counts > 0, 1.0 / (counts * HW), 0.0).astype(np.float32)
    aux = np.zeros((P, G + G + NT), np.float32)
    aux[:, 0:G] = np.arange(G, dtype=np.float32)[None, :]
    aux[:, G:2 * G] = scale[None, :]
    aux[:, 2 * G:] = idx.reshape(NT, P).T.astype(np.float32)
    xr = x.reshape(N, M, HW)
    in_maps = []
    for k in range(CORES):
        shard = np.ascontiguousarray(xr[:, k * ML:(k + 1) * ML, :]).reshape(N, F)
        in_maps.append({"x": shard, "aux": aux})
    return in_maps


def run(x, tensor_list_assignmentindices, trace=False):
    in_maps = _prepare(x, tensor_list_assignmentindices)
    nc = _build()
    res = run_bass_kernel_spmd(nc, in_maps, core_ids=list(range(CORES)), trace=trace)
    outs = [np.asarray(r["out"]) for r in res.results]
    out = np.concatenate(outs, axis=1)  # [G, M]
    return out.reshape(G, M, 1, 1).astype(np.float32), res.exec_time_ns


def kernel(**inputs):
    out, _ = run(inputs["x"], inputs["tensor_list_assignmentindices"], trace=False)
    return out


# revision 30
# speedup vs baseline: 1.0331x; 1.0331x over previous
"""Trainium2 Bass kernel for nn_AvgPoolVectorsPerWSI (segment-mean over groups).

Math: x [N=2048, M=512, 7, 7], idx [N] in [0,64)
  out[g, m] = mean over {n: idx[n]==g} and spatial of x[n, m, :, :]  -> [64, 512, 1, 1]

Strategy (no collectives needed):
  - Shard over M: core k handles an m-slice of 64 channels. Each core
    streams its x slice [2048, 64, 49] (25.7 MB) once at fabric line rate
    (~430 GB/s, 466 ns per 12.5 KB row descriptor); the ~62 us stream is
    the floor and everything else hides behind it. 16 n-tiles of 128 rows
    (128-partition DMAs only -- partition-partial bulk DMAs mis-lower).
  - Per tile: PE accumulates the first MC=10 channels raw
    (psum_raw[g, (m,j)] += w^T @ x, one 490-col fp32 chunk), DVE spatially
    j-reduces the other 54 channels ([128, 54, 49] -> [128, 54],
    ~2.9 us/tile vs ~3.9 us/tile DMA pace -- real headroom), then PE adds
    two small fp32 matmuls psum[g, m] += w^T @ xs, split m[10:46]/m[46:64]
    so only a sliver depends on the last DMA piece. w is the
    scale-weighted one-hot (scale = 1/(count_g*49)) generated ON DEVICE
    from a 74 KB aux tensor loaded first on the ring (w-gen and the whole
    DVE chain hang off it).
  - Tail minimization (stream end -> block end is what matters; the ~7 us
    NRT postamble after it is fixed):
      * tile 15 arrives in 4 column pieces (raw chunk, then 3x18 channels)
        so the last exposed reduce is ~1 us;
      * PSUM->SBUF copies run on ScalarE, warmed up at start (the first
        activation op lazily loads the ACT table, ~1.3 us);
      * DVE j-reduces psum_raw into out[:, 0:10] concurrently with the
        final matmul sliver;
      * the final out DMA has no completion wait -- NRT's postamble
        (sem resets, then dma_rearm) drains the ring with ~5 us margin.

Raw Block implementation (not Tile): the walrus matmul/DMA lowerings only
accept ONE attached sync-wait per instruction; standalone wait_ge
instructions sidestep that.

PSUM: a matmul with start=True clears the has_written bits of its WHOLE
bank, so the three accumulation streams (raw / half A / half B) live in
three separate psum tensors.

Known environmental hazard: SDMA engine 15 episodically runs ~20-25% slow,
and every tile's completion semaphore needs all 16 engines, so on such
runs the stream is engine-15-paced (~+14 us) regardless of kernel
structure; partition-exclusion workarounds mis-lower (see above), so this
is accepted as run-to-run variance.
"""

from contextlib import ExitStack

import numpy as np

import concourse.bass as bass
import concourse.mybir as mybir
from concourse.bass_utils import run_bass_kernel_spmd

N = 2048          # samples
M = 512           # channels
HW = 49           # spatial (7*7)
G = 64            # groups
CORES = 8
ML = M // CORES   # 64 channels per core
F = ML * HW       # 3136 floats per (n, core)
P = 128           # partitions per tile
NT = N // P       # 16 n-tiles
BUFS = 8          # x slot ring depth == number of DMA semaphores
MC = 10           # channels on the PE raw path
FC = MC * HW      # 490 raw columns (single PSUM bank)
MV = ML - MC      # 54 channels on the DVE reduce path
# tile 15 pieces: raw chunk [0:FC], then channel blocks tapered so the last
# exposed reduce (and the matmul sliver behind it) is small. Finer splits
# measured worse: on slow-engine runs the straggler delivers all pieces
# bunched at its stream end, so extra pieces only add receipt hops/drains.
PIECE_CH = [(MC, 30), (30, 56), (56, 64)]
NP = 1 + len(PIECE_CH)  # piece DMAs incl. the raw chunk
MB = 56           # matmul half A covers m[MC:56]; half B covers m[56:64]

F32 = mybir.dt.float32


def _build():
    nc = bass.Bass(trn_type="TRN2", target_bir_lowering=False)
    x_ext = nc.declare_dram_parameter("x", [N, F], F32, isOutput=False)
    # aux[:, 0:64] iota row, aux[:, 64:128] scale row, aux[:, 128:144] idx
    aux_ext = nc.declare_dram_parameter("aux", [P, G + G + NT], F32,
                                        isOutput=False)
    out_ext = nc.declare_dram_parameter("out", [G, ML], F32, isOutput=True)

    x_t = x_ext.ap().rearrange("(t p) f -> t p f", p=P)  # [16, 128, 3136]

    with ExitStack() as ctx:
        x_buf = ctx.enter_context(nc.sbuf_tensor([P, BUFS * F], F32))
        xs_buf = ctx.enter_context(nc.sbuf_tensor([P, BUFS * MV], F32))
        aux_sb = ctx.enter_context(nc.sbuf_tensor([P, G + G + NT], F32))
        w_sb = ctx.enter_context(nc.sbuf_tensor([P, NT * G], F32))
        out_sb = ctx.enter_context(nc.sbuf_tensor([G, ML], F32))
        warm_sb = ctx.enter_context(nc.sbuf_tensor([P, 1], F32))
        # separate tensors -> separate PSUM accumulation state
        psum_raw = ctx.enter_context(nc.psum_tensor([G, FC], F32))
        psum_a = ctx.enter_context(nc.psum_tensor([G, MB - MC], F32))
        psum_b = ctx.enter_context(nc.psum_tensor([G, ML - MB], F32))
        dma_x = [
            ctx.enter_context(nc.semaphore(name=f"dma_x{s}"))
            for s in range(BUFS)
        ]
        # one dedicated sem per tile-15 piece: cumulative thresholds on a
        # shared sem alias across outstanding transfers (a straggling SDMA
        # engine's missing stripes can be masked by other engines' incs
        # from later pieces), so each piece gets a single-use sem
        dma_p = [
            ctx.enter_context(nc.semaphore(name=f"dma_p{p}"))
            for p in range(NP)
        ]
        # tile 0's second row-half (its own sem: two transfers may be
        # outstanding at once, and only an exact-total threshold is
        # alias-safe)
        dma_h = ctx.enter_context(nc.semaphore(name="dma_h"))
        dma_a = ctx.enter_context(nc.semaphore())   # +16 when aux resident
        dma_o = ctx.enter_context(nc.semaphore())   # out DMA (never waited)
        wg_sem = ctx.enter_context(nc.semaphore())  # +1 when w generated
        red_sem = ctx.enter_context(nc.semaphore())  # +1 per DVE reduce op
        pe_sem = ctx.enter_context(nc.semaphore())   # +1 per PE matmul
        fin_sem = ctx.enter_context(nc.semaphore())  # +1 per out_sb writer
        block = ctx.enter_context(nc.Block())

        # cumulative dma_x thresholds: slot s serves tiles s and s+8;
        # tile 15 (slot 7) arrives as 4 pieces of +16 each (32/48/64/80)
        def xthresh(t):
            return 16 * (t // BUFS) + 16

        # pe_sem incs: 3 per tile (raw, mm_a, mm_b)
        def pe_count(t):
            return 3 * (t + 1)

        # ---- x-stream DMA program (SP / HWDGE ring, FIFO) ----
        @block.sync
        def _(sync):
            # tile 0 in two row-halves: 64 descriptors generate in half the
            # time, so the first HBM byte (and the whole stream) moves up
            sync.dma_start(
                out=x_buf[0:64, 0:F], in_=x_t[0][0:64, :]
            ).then_inc(dma_x[0], 16)
            sync.dma_start(
                out=x_buf[64:128, 0:F], in_=x_t[0][64:128, :]
            ).then_inc(dma_h, 16)
            # aux after tile 0: its descriptor gen overlaps tile 0's
            # stream instead of delaying the first x byte; the hybrid's
            # DVE slack absorbs the later w-gen start
            sync.dma_start(out=aux_sb[:, :], in_=aux_ext.ap()).then_inc(
                dma_a, 16
            )
            for t in range(1, NT - 1):
                if t >= BUFS:
                    # slot reuse: all three matmuls of tile t-8 are done
                    # (mm_b implies the DVE reduce consumed the slot too)
                    sync.wait_ge(pe_sem, pe_count(t - BUFS))
                slot = t % BUFS
                sync.dma_start(
                    out=x_buf[:, slot * F:(slot + 1) * F], in_=x_t[t]
                ).then_inc(dma_x[slot], 16)
            # tile 15 -> slot 7, 4 column pieces
            sync.wait_ge(pe_sem, pe_count(NT - 1 - BUFS))
            bounds = [0, FC] + [hi * HW for _, hi in PIECE_CH]
            for p in range(NP):
                lo, hi = bounds[p], bounds[p + 1]
                sync.dma_start(
                    out=x_buf[:, 7 * F + lo:7 * F + hi],
                    in_=x_t[NT - 1][:, lo:hi],
                ).then_inc(dma_p[p], 16)
            # (the out DMA is issued from the ACT queue right after its
            # copies -- saves a cross-engine semaphore hop)

        # ---- VectorE: w generation, spatial j-reduction, raw epilogue ----
        @block.vector
        def _(vector):
            # w[p, t*G+g] = (idx[t*128+p] == g) * scale[g]
            vector.wait_ge(dma_a, 16)
            for t in range(NT):
                wg = vector.scalar_tensor_tensor(
                    out=w_sb[:, t * G:(t + 1) * G],
                    in0=aux_sb[:, 0:G],
                    scalar=aux_sb[:, 2 * G + t:2 * G + t + 1],
                    in1=aux_sb[:, G:2 * G],
                    op0=mybir.AluOpType.is_equal,
                    op1=mybir.AluOpType.mult,
                )
            wg.then_inc(wg_sem, 1)

            for t in range(NT - 1):
                if t >= BUFS:
                    # xs slot reuse: tile t-8's matmuls consumed it
                    vector.wait_ge(pe_sem, pe_count(t - BUFS))
                vector.wait_ge(dma_x[t % BUFS], xthresh(t))
                if t == 0:
                    vector.wait_ge(dma_h, 16)
                slot = t % BUFS
                vector.tensor_reduce(
                    out=xs_buf[:, slot * MV:(slot + 1) * MV],
                    in_=x_buf[:, slot * F + FC:(slot + 1) * F]
                    .rearrange("p (m j) -> p m j", j=HW),
                    axis=mybir.AxisListType.X,
                    op=mybir.AluOpType.add,
                ).then_inc(red_sem, 1)
            # tile 15 (slot 7): 3 piece reduces of 18 channels
            vector.wait_ge(pe_sem, pe_count(NT - 1 - BUFS))
            for p, (clo, chi) in enumerate(PIECE_CH):
                vector.wait_ge(dma_p[p + 1], 16)
                vector.tensor_reduce(
                    out=xs_buf[:, 7 * MV + (clo - MC):7 * MV + (chi - MC)],
                    in_=x_buf[:, 7 * F + clo * HW:7 * F + chi * HW]
                    .rearrange("p (m j) -> p m j", j=HW),
                    axis=mybir.AxisListType.X,
                    op=mybir.AluOpType.add,
                ).then_inc(red_sem, 1)
            # epilogue: j-reduce the raw-path PSUM into out[:, 0:MC]
            vector.wait_ge(pe_sem, pe_count(NT - 2) + 1)  # raw(15) done
            vector.tensor_reduce(
                out=out_sb[:, 0:MC],
                in_=psum_raw[:, :].rearrange("p (m j) -> p m j", j=HW),
                axis=mybir.AxisListType.X,
                op=mybir.AluOpType.add,
            ).then_inc(fin_sem, 1)

        # ---- TensorE: raw chunk + two xs matmuls per tile (fp32) ----
        @block.tensor
        def _(tensor):
            tensor.wait_ge(wg_sem, 1)
            for t in range(NT):
                slot = t % BUFS
                wt = w_sb[:, t * G:(t + 1) * G]
                first, last = (t == 0), (t == NT - 1)
                # raw chunk needs only the tile's first piece in DMA order
                if last:
                    tensor.wait_ge(dma_p[0], 16)
                else:
                    tensor.wait_ge(dma_x[slot], xthresh(t))
                    if first:
                        tensor.wait_ge(dma_h, 16)
                tensor.matmul(
                    out=psum_raw[:, :],
                    lhsT=wt,
                    rhs=x_buf[:, slot * F:slot * F + FC],
                    start=first,
                    stop=last,
                ).then_inc(pe_sem, 1)
                # xs halves; tile 15's reduces land piecewise: half A
                # needs all but the last piece, half B needs the last
                tensor.wait_ge(
                    red_sem, (t + 1) + (len(PIECE_CH) - 2 if last else 0)
                )
                tensor.matmul(
                    out=psum_a[:, :],
                    lhsT=wt,
                    rhs=xs_buf[:, slot * MV:slot * MV + (MB - MC)],
                    start=first,
                    stop=last,
                ).then_inc(pe_sem, 1)
                if last:
                    tensor.wait_ge(red_sem, (t + 1) + len(PIECE_CH) - 1)
                tensor.matmul(
                    out=psum_b[:, :],
                    lhsT=wt,
                    rhs=xs_buf[:, slot * MV + (MB - MC):(slot + 1) * MV],
                    start=first,
                    stop=last,
                ).then_inc(pe_sem, 1)

        # ---- ScalarE (ACT): PSUM -> SBUF copies ----
        @block.scalar
        def _(scalar):
            # warmup: the first activation op lazily DMAs the ACT function
            # table (~1.3 us); trigger it off the critical path
            scalar.activation(
                out=warm_sb[:, :],
                in_=warm_sb[:, :],
                func=mybir.ActivationFunctionType.Copy,
            )
            scalar.wait_ge(pe_sem, pe_count(NT - 1) - 1)  # mm_a(15)
            scalar.activation(
                out=out_sb[:, MC:MB],
                in_=psum_a[:, :],
                func=mybir.ActivationFunctionType.Copy,
            )
            scalar.wait_ge(pe_sem, pe_count(NT - 1))      # mm_b(15)
            scalar.activation(
                out=out_sb[:, MB:ML],
                in_=psum_b[:, :],
                func=mybir.ActivationFunctionType.Copy,
            )
            # out: issue right here (program order covers the two copies;
            # fin covers DVE's raw j-reduce) and do NOT wait -- NRT's
            # postamble drains the ring before rearm
            scalar.wait_ge(fin_sem, 1)
            scalar.dma_start(out=out_ext.ap(), in_=out_sb[:, :]).then_inc(
                dma_o, 16
            )

    return nc


def _prepare(x, idx):
    x = np.asarray(x)
    if x.dtype != np.float32:
        x = x.astype(np.float32)
    idx = np.asarray(idx).astype(np.int64)
    counts = np.bincount(idx, minlength=G).astype(np.float64)
    scale = np.where(counts > 0, 1.0 / (counts * HW), 0.0).astype(np.float32)
    aux = np.zeros((P, G + G + NT), np.float32)
    aux[:, 0:G] = np.arange(G, dtype=np.float32)[None, :]
    aux[:, G:2 * G] = scale[None, :]
    aux[:, 2 * G:] = idx.reshape(NT, P).T.astype(np.float32)
    xr = x.reshape(N, M, HW)
    in_maps = []
    for k in range(CORES):
        shard = np.ascontiguousarray(xr[:, k * ML:(k + 1) * ML, :]).reshape(N, F)
        in_maps.append({"x": shard, "aux": aux})
    return in_maps


def run(x, tensor_list_assignmentindices, trace=False):
    in_maps = _prepare(x, tensor_list_assignmentindices)
    nc = _build()
    res = run_bass_kernel_spmd(nc, in_maps, core_ids=list(range(CORES)), trace=trace)
    outs = [np.asarray(r["out"]) for r in res.results]
    out = np.concatenate(outs, axis=1)  # [G, M]
    return out.reshape(G, M, 1, 1).astype(np.float32), res.exec_time_ns


def kernel(**inputs):
    out, _ = run(inputs["x"], inputs["tensor_list_assignmentindices"], trace=False)
    return out


# revision 31
# speedup vs baseline: 1.1646x; 1.1273x over previous
"""Trainium2 Bass kernel for nn_AvgPoolVectorsPerWSI (segment-mean over groups).

Math: x [N=2048, M=512, 7, 7], idx [N] in [0,64)
  out[g, m] = mean over {n: idx[n]==g} and spatial of x[n, m, :, :]  -> [64, 512, 1, 1]

Strategy (no collectives needed):
  - Shard over M: core k handles an m-slice of 64 channels. Each core
    streams its x slice [2048, 64, 49] (25.7 MB) once at fabric line rate
    (~430 GB/s, 466 ns per 12.5 KB row descriptor); the ~62 us stream is
    the floor and everything else hides behind it. 16 n-tiles of 128 rows
    (128-partition DMAs only -- partition-partial bulk DMAs mis-lower).
  - Per tile: PE accumulates the first MC=10 channels raw
    (psum_raw[g, (m,j)] += w^T @ x, one 490-col fp32 chunk), DVE spatially
    j-reduces the other 54 channels ([128, 54, 49] -> [128, 54],
    ~2.9 us/tile vs ~3.9 us/tile DMA pace -- real headroom), then PE adds
    two small fp32 matmuls psum[g, m] += w^T @ xs, split m[10:46]/m[46:64]
    so only a sliver depends on the last DMA piece. w is the
    scale-weighted one-hot (scale = 1/(count_g*49)) generated ON DEVICE
    from a 74 KB aux tensor loaded first on the ring (w-gen and the whole
    DVE chain hang off it).
  - Tail minimization (stream end -> block end is what matters; the ~7 us
    NRT postamble after it is fixed):
      * tile 15 arrives in 4 column pieces (raw chunk, then 3x18 channels)
        so the last exposed reduce is ~1 us;
      * PSUM->SBUF copies run on ScalarE, warmed up at start (the first
        activation op lazily loads the ACT table, ~1.3 us);
      * DVE j-reduces psum_raw into out[:, 0:10] concurrently with the
        final matmul sliver;
      * the final out DMA has no completion wait -- NRT's postamble
        (sem resets, then dma_rearm) drains the ring with ~5 us margin.

Raw Block implementation (not Tile): the walrus matmul/DMA lowerings only
accept ONE attached sync-wait per instruction; standalone wait_ge
instructions sidestep that.

PSUM: a matmul with start=True clears the has_written bits of its WHOLE
bank, so the three accumulation streams (raw / half A / half B) live in
three separate psum tensors.

Known environmental hazard: SDMA engine 15 episodically runs ~20-25% slow,
and every tile's completion semaphore needs all 16 engines, so on such
runs the stream is engine-15-paced (~+14 us) regardless of kernel
structure; partition-exclusion workarounds mis-lower (see above), so this
is accepted as run-to-run variance.
"""

from contextlib import ExitStack

import numpy as np

import concourse.bass as bass
import concourse.mybir as mybir
from concourse.bass_utils import run_bass_kernel_spmd

N = 2048          # samples
M = 512           # channels
HW = 49           # spatial (7*7)
G = 64            # groups
CORES = 8
ML = M // CORES   # 64 channels per core
F = ML * HW       # 3136 floats per (n, core)
P = 128           # partitions per tile
NT = N // P       # 16 n-tiles
BUFS = 8          # x slot ring depth == number of DMA semaphores
MC = 10           # channels on the PE raw path
FC = MC * HW      # 490 raw columns (single PSUM bank)
MV = ML - MC      # 54 channels on the DVE reduce path
# tile 15 pieces: raw chunk [0:FC], then 4 near-equal channel blocks.
# Sizing balances the straggler-run chains delivery(k) + remaining reduces
# across k: coarser (3 tapered) exposes a big middle reduce, finer (6x9)
# just adds receipt hops/drains since a slow engine delivers all pieces
# bunched at its stream end.
PIECE_CH = [(MC, 24), (24, 38), (38, 52), (52, 64)]
NP = 1 + len(PIECE_CH)  # piece DMAs incl. the raw chunk
MB = 52           # matmul half A covers m[MC:52]; half B covers m[52:64]

F32 = mybir.dt.float32


def _build():
    nc = bass.Bass(trn_type="TRN2", target_bir_lowering=False)
    x_ext = nc.declare_dram_parameter("x", [N, F], F32, isOutput=False)
    # aux[:, 0:64] iota row, aux[:, 64:128] scale row, aux[:, 128:144] idx
    aux_ext = nc.declare_dram_parameter("aux", [P, G + G + NT], F32,
                                        isOutput=False)
    out_ext = nc.declare_dram_parameter("out", [G, ML], F32, isOutput=True)

    x_t = x_ext.ap().rearrange("(t p) f -> t p f", p=P)  # [16, 128, 3136]

    with ExitStack() as ctx:
        x_buf = ctx.enter_context(nc.sbuf_tensor([P, BUFS * F], F32))
        xs_buf = ctx.enter_context(nc.sbuf_tensor([P, BUFS * MV], F32))
        aux_sb = ctx.enter_context(nc.sbuf_tensor([P, G + G + NT], F32))
        w_sb = ctx.enter_context(nc.sbuf_tensor([P, NT * G], F32))
        out_sb = ctx.enter_context(nc.sbuf_tensor([G, ML], F32))
        warm_sb = ctx.enter_context(nc.sbuf_tensor([P, 1], F32))
        # separate tensors -> separate PSUM accumulation state
        psum_raw = ctx.enter_context(nc.psum_tensor([G, FC], F32))
        psum_a = ctx.enter_context(nc.psum_tensor([G, MB - MC], F32))
        psum_b = ctx.enter_context(nc.psum_tensor([G, ML - MB], F32))
        dma_x = [
            ctx.enter_context(nc.semaphore(name=f"dma_x{s}"))
            for s in range(BUFS)
        ]
        # one dedicated sem per tile-15 piece: cumulative thresholds on a
        # shared sem alias across outstanding transfers (a straggling SDMA
        # engine's missing stripes can be masked by other engines' incs
        # from later pieces), so each piece gets a single-use sem
        dma_p = [
            ctx.enter_context(nc.semaphore(name=f"dma_p{p}"))
            for p in range(NP)
        ]
        # tile 0's second row-half (its own sem: two transfers may be
        # outstanding at once, and only an exact-total threshold is
        # alias-safe)
        dma_h = ctx.enter_context(nc.semaphore(name="dma_h"))
        dma_a = ctx.enter_context(nc.semaphore())   # +16 when aux resident
        dma_o = ctx.enter_context(nc.semaphore())   # out DMA (never waited)
        wg_sem = ctx.enter_context(nc.semaphore())  # +1 when w generated
        red_sem = ctx.enter_context(nc.semaphore())  # +1 per DVE reduce op
        pe_sem = ctx.enter_context(nc.semaphore())   # +1 per PE matmul
        fin_sem = ctx.enter_context(nc.semaphore())  # +1 per out_sb writer
        block = ctx.enter_context(nc.Block())

        # cumulative dma_x thresholds: slot s serves tiles s and s+8;
        # tile 15 (slot 7) arrives as 4 pieces of +16 each (32/48/64/80)
        def xthresh(t):
            return 16 * (t // BUFS) + 16

        # pe_sem incs: 3 per tile (raw, mm_a, mm_b)
        def pe_count(t):
            return 3 * (t + 1)

        # ---- x-stream DMA program (SP / HWDGE ring, FIFO) ----
        @block.sync
        def _(sync):
            # tile 0 in two row-halves: 64 descriptors generate in half the
            # time, so the first HBM byte (and the whole stream) moves up
            sync.dma_start(
                out=x_buf[0:64, 0:F], in_=x_t[0][0:64, :]
            ).then_inc(dma_x[0], 16)
            sync.dma_start(
                out=x_buf[64:128, 0:F], in_=x_t[0][64:128, :]
            ).then_inc(dma_h, 16)
            # aux after tile 0: its descriptor gen overlaps tile 0's
            # stream instead of delaying the first x byte; the hybrid's
            # DVE slack absorbs the later w-gen start
            sync.dma_start(out=aux_sb[:, :], in_=aux_ext.ap()).then_inc(
                dma_a, 16
            )
            for t in range(1, NT - 1):
                if t >= BUFS:
                    # slot reuse: all three matmuls of tile t-8 are done
                    # (mm_b implies the DVE reduce consumed the slot too)
                    sync.wait_ge(pe_sem, pe_count(t - BUFS))
                slot = t % BUFS
                sync.dma_start(
                    out=x_buf[:, slot * F:(slot + 1) * F], in_=x_t[t]
                ).then_inc(dma_x[slot], 16)
            # tile 15 -> slot 7, 4 column pieces
            sync.wait_ge(pe_sem, pe_count(NT - 1 - BUFS))
            bounds = [0, FC] + [hi * HW for _, hi in PIECE_CH]
            for p in range(NP):
                lo, hi = bounds[p], bounds[p + 1]
                sync.dma_start(
                    out=x_buf[:, 7 * F + lo:7 * F + hi],
                    in_=x_t[NT - 1][:, lo:hi],
                ).then_inc(dma_p[p], 16)
            # (the out DMA is issued from the ACT queue right after its
            # copies -- saves a cross-engine semaphore hop)

        # ---- VectorE: w generation, spatial j-reduction, raw epilogue ----
        @block.vector
        def _(vector):
            # w[p, t*G+g] = (idx[t*128+p] == g) * scale[g]
            vector.wait_ge(dma_a, 16)
            for t in range(NT):
                wg = vector.scalar_tensor_tensor(
                    out=w_sb[:, t * G:(t + 1) * G],
                    in0=aux_sb[:, 0:G],
                    scalar=aux_sb[:, 2 * G + t:2 * G + t + 1],
                    in1=aux_sb[:, G:2 * G],
                    op0=mybir.AluOpType.is_equal,
                    op1=mybir.AluOpType.mult,
                )
            wg.then_inc(wg_sem, 1)

            for t in range(NT - 1):
                if t >= BUFS:
                    # xs slot reuse: tile t-8's matmuls consumed it
                    vector.wait_ge(pe_sem, pe_count(t - BUFS))
                vector.wait_ge(dma_x[t % BUFS], xthresh(t))
                if t == 0:
                    vector.wait_ge(dma_h, 16)
                slot = t % BUFS
                vector.tensor_reduce(
                    out=xs_buf[:, slot * MV:(slot + 1) * MV],
                    in_=x_buf[:, slot * F + FC:(slot + 1) * F]
                    .rearrange("p (m j) -> p m j", j=HW),
                    axis=mybir.AxisListType.X,
                    op=mybir.AluOpType.add,
                ).then_inc(red_sem, 1)
            # tile 15 (slot 7): 3 piece reduces of 18 channels
            vector.wait_ge(pe_sem, pe_count(NT - 1 - BUFS))
            for p, (clo, chi) in enumerate(PIECE_CH):
                vector.wait_ge(dma_p[p + 1], 16)
                vector.tensor_reduce(
                    out=xs_buf[:, 7 * MV + (clo - MC):7 * MV + (chi - MC)],
                    in_=x_buf[:, 7 * F + clo * HW:7 * F + chi * HW]
                    .rearrange("p (m j) -> p m j", j=HW),
                    axis=mybir.AxisListType.X,
                    op=mybir.AluOpType.add,
                ).then_inc(red_sem, 1)
            # epilogue: j-reduce the raw-path PSUM into out[:, 0:MC]
            vector.wait_ge(pe_sem, pe_count(NT - 2) + 1)  # raw(15) done
            vector.tensor_reduce(
                out=out_sb[:, 0:MC],
                in_=psum_raw[:, :].rearrange("p (m j) -> p m j", j=HW),
                axis=mybir.AxisListType.X,
                op=mybir.AluOpType.add,
            ).then_inc(fin_sem, 1)

        # ---- TensorE: raw chunk + two xs matmuls per tile (fp32) ----
        @block.tensor
        def _(tensor):
            tensor.wait_ge(wg_sem, 1)
            for t in range(NT):
                slot = t % BUFS
                wt = w_sb[:, t * G:(t + 1) * G]
                first, last = (t == 0), (t == NT - 1)
                # raw chunk needs only the tile's first piece in DMA order
                if last:
                    tensor.wait_ge(dma_p[0], 16)
                else:
                    tensor.wait_ge(dma_x[slot], xthresh(t))
                    if first:
                        tensor.wait_ge(dma_h, 16)
                tensor.matmul(
                    out=psum_raw[:, :],
                    lhsT=wt,
                    rhs=x_buf[:, slot * F:slot * F + FC],
                    start=first,
                    stop=last,
                ).then_inc(pe_sem, 1)
                # xs halves; tile 15's reduces land piecewise: half A
                # needs all but the last piece, half B needs the last
                tensor.wait_ge(
                    red_sem, (t + 1) + (len(PIECE_CH) - 2 if last else 0)
                )
                tensor.matmul(
                    out=psum_a[:, :],
                    lhsT=wt,
                    rhs=xs_buf[:, slot * MV:slot * MV + (MB - MC)],
                    start=first,
                    stop=last,
                ).then_inc(pe_sem, 1)
                if last:
                    tensor.wait_ge(red_sem, (t + 1) + len(PIECE_CH) - 1)
                tensor.matmul(
                    out=psum_b[:, :],
                    lhsT=wt,
                    rhs=xs_buf[:, slot * MV + (MB - MC):(slot + 1) * MV],
                    start=first,
                    stop=last,
                ).then_inc(pe_sem, 1)

        # ---- ScalarE (ACT): PSUM -> SBUF copies ----
        @block.scalar
        def _(scalar):
            # warmup: the first activation op lazily DMAs the ACT function
            # table (~1.3 us); trigger it off the critical path
            scalar.activation(
                out=warm_sb[:, :],
                in_=warm_sb[:, :],
                func=mybir.ActivationFunctionType.Copy,
            )
            scalar.wait_ge(pe_sem, pe_count(NT - 1) - 1)  # mm_a(15)
            scalar.activation(
                out=out_sb[:, MC:MB],
                in_=psum_a[:, :],
                func=mybir.ActivationFunctionType.Copy,
            )
            scalar.wait_ge(pe_sem, pe_count(NT - 1))      # mm_b(15)
            scalar.activation(
                out=out_sb[:, MB:ML],
                in_=psum_b[:, :],
                func=mybir.ActivationFunctionType.Copy,
            )
            # out: issue right here (program order covers the two copies;
            # fin covers DVE's raw j-reduce) and do NOT wait -- NRT's
            # postamble drains the ring before rearm
            scalar.wait_ge(fin_sem, 1)
            scalar.dma_start(out=out_ext.ap(), in_=out_sb[:, :]).then_inc(
                dma_o, 16
            )

    return nc


def _prepare(x, idx):
    x = np.asarray(x)
    if x.dtype != np.float32:
        x = x.astype(np.float32)
    idx = np.asarray(idx).astype(np.int64)
    counts = np.bincount(idx, minlength=G).astype(np.float64)
    scale = np.where(counts > 0, 1.0 / (counts * HW), 0.0).astype(np.float32)
    aux = np.zeros((P, G + G + NT), np.float32)
    aux[:, 0:G] = np.arange(G, dtype=np.float32)[None, :]
    aux[:, G:2 * G] = scale[None, :]
    aux[:, 2 * G:] = idx.reshape(NT, P).T.astype(np.float32)
    xr = x.reshape(N, M, HW)
    in_maps = []
    for k in range(CORES):
        shard = np.ascontiguousarray(xr[:, k * ML:(k + 1) * ML, :]).reshape(N, F)
        in_maps.append({"x": shard, "aux": aux})
    return in_maps


def run(x, tensor_list_assignmentindices, trace=False):
    in_maps = _prepare(x, tensor_list_assignmentindices)
    nc = _build()
    res = run_bass_kernel_spmd(nc, in_maps, core_ids=list(range(CORES)), trace=trace)
    outs = [np.asarray(r["out"]) for r in res.results]
    out = np.concatenate(outs, axis=1)  # [G, M]
    return out.reshape(G, M, 1, 1).astype(np.float32), res.exec_time_ns


def kernel(**inputs):
    out, _ = run(inputs["x"], inputs["tensor_list_assignmentindices"], trace=False)
    return out
